# revision 45
# baseline (speedup 1.0000x reference)
"""Trainium2 Bass kernel for nn_DistinctivenessLoss.

Math: with unit-normalized 48-dim descriptors, dist(a,b) < 1  <=>  <a,b> > 0.5,
so each image's two 4096x4096 cdist+count reductions collapse to ONE 4096x4096
gram matrix S = d1^T d2 per image:
  mx1[i] = #{j : S[i,j] > 0.5}          (row counts)
  mx2[i] = colcount[flat2[i]],  colcount[j] = #{i : S[i,j] > 0.5}

Sharding: 2 cores per image (B=4, 8 cores); each core computes 2048 rows of S.
On-device per core:
  - gram matmuls in bf16 (inputs rounded to bf16; fp32 PSUM accumulate --
    measured end-to-end loss impact ~4e-5 relative), row-tiled 2x (K=48<=64)
    at tile positions (0,0)/(64,0)
  - threshold pass split between ScalarE (Sign activation, fused row-sum via
    accum_out) and VectorE (tensor_scalar is_gt, fused row-sum) over 2-bank
    [128,1024] PSUM supertiles, emitting bf16 sign/indicator tiles
  - column counts via PE "selector" matmuls over the sign tiles (weight
    column c routes chunk c's colsum into PSUM row c), accumulated in 2 banks
  - the peaky-loss pooling for one 64x64 attention map (avg pools as banded
    matmuls, 33x33 max pool as log-step max chains + 32x32 DVE transposes)
Host: input normalization/layout, count decode, gathers, and the O(n) loss
epilogue (exact fp32, negligible work).
"""

import numpy as np
import ml_dtypes

B, C, H, W = 4, 48, 64, 64
N = H * W                  # 4096
NCORES = 8
ROWS = N // 2              # rows of S per core
NT = ROWS // 128           # 16 row-tiles per core
NCH = N // 512             # 8 column chunks
NSC = NCH // 2             # 4 superchunks (2-bank PSUM supertiles)
THR = 0.5
TAU = 0.25
LAMBDA_PEAKY = 0.2
NEG_INF = float("-inf")

# (row-tile, superchunk) -> engine class: 0=ACT/sign, 1=DVE/is_gt. Greedy
# balance using measured per-supertile costs plus each engine's fixed work
# (peaky chains/memsets on DVE, copies/table-load on ACT).
ASSIGN = np.fromfunction(lambda t, s: (t + s) % 2, (NT, NSC), dtype=np.int64)
N_ACT_PER_CHUNK = np.array([(ASSIGN[:, c // 2] == 0).sum() for c in range(NCH)])
N_ACT_PER_TILE = (ASSIGN == 0).sum(axis=1)    # [NT], in superchunk units

_PROGRAM = None


def _build_program():
    from contextlib import ExitStack

    import concourse.bass as bass
    import concourse.mybir as mybir
    import concourse.tile as tile
    from concourse import bacc

    F32 = mybir.dt.float32
    BF16 = mybir.dt.bfloat16
    Sign = mybir.ActivationFunctionType.Sign
    ts, ds = bass.ts, bass.ds

    nc = bacc.Bacc("TRN2", target_bir_lowering=False, debug=False,
                   num_devices=NCORES)

    d1w_d = nc.dram_tensor("d1w", [128, NT * 64], BF16, kind="ExternalInput").ap()
    d2d_d = nc.dram_tensor("d2d", [128, N], BF16, kind="ExternalInput").ap()
    wsel_d = nc.dram_tensor("wsel", [128, NCH * 2 * 128], BF16,
                            kind="ExternalInput").ap()
    attn_d = nc.dram_tensor("attn", [64, 64], F32, kind="ExternalInput").ap()
    pk_d = nc.dram_tensor("pk", [64, 128], F32, kind="ExternalInput").ap()

    rowA_d = nc.dram_tensor("rowA", [128, NT], F32, kind="ExternalOutput").ap()
    rowD_d = nc.dram_tensor("rowD", [128, NT], F32, kind="ExternalOutput").ap()
    cols_d = nc.dram_tensor("cols", [NCH, 512], F32, kind="ExternalOutput").ap()
    pkout_d = nc.dram_tensor("pkout", [64, 2], F32, kind="ExternalOutput").ap()

    with tile.TileContext(nc) as tc, ExitStack() as ctx:
        sb = ctx.enter_context(tc.tile_pool(name="sb", bufs=1))
        sgn_pool = ctx.enter_context(tc.tile_pool(name="sgn", bufs=3))

        # ---- inputs: peaky constants first (peaky matmuls head the PE queue,
        # so their inputs gate everything), then gram operands by first use ----
        pk = sb.tile([64, 128], F32)
        nc.sync.dma_start(pk[:], pk_d)
        attn = sb.tile([64, 128], F32)
        nc.gpsimd.memset(attn[:, 64:128], 0.0)
        nc.sync.dma_start(attn[0:64, 0:64], attn_d)

        d1w = sb.tile([128, NT * 64], BF16)
        d2d = sb.tile([128, N], BF16)
        nc.sync.dma_start(d1w[:, 0:128], d1w_d[:, 0:128])
        nc.sync.dma_start(d2d[:, 0:1024], d2d_d[:, 0:1024])
        nc.sync.dma_start(d1w[:, 128:1024], d1w_d[:, 128:1024])
        wsel = sb.tile([128, NCH * 2 * 128], BF16)
        nc.sync.dma_start(wsel[:, 0:512], wsel_d[:, 0:512])
        nc.sync.dma_start(d2d[:, 1024:2048], d2d_d[:, 1024:2048])
        nc.sync.dma_start(d2d[:, 2048:4096], d2d_d[:, 2048:4096])
        nc.sync.dma_start(wsel[:, 512:2048], wsel_d[:, 512:2048])

        nthr = sb.tile([128, 1], F32)
        nc.vector.memset(nthr[:], -THR)

        rowaccA = sb.tile([128, NT * NSC], F32)
        rowaccD = sb.tile([128, NT * NSC], F32)
        nc.vector.memset(rowaccA[:], 0.0)
        nc.vector.memset(rowaccD[:], 0.0)

        # ---- peaky loss first: fills the input-DMA bubble + warms the PE.
        # Its PSUM pool is scoped so the bank is returned before the main
        # pools claim all 8 banks. ----
        pk_scope = ExitStack()
        pk_pool = pk_scope.enter_context(
            tc.tile_pool(name="pkps", bufs=1, space="PSUM"))

        # PE warm-up: dummy matmuls with no DMA dependencies run during the
        # input-DMA window and flip the HAM clock gate (1.2 -> 2.4 GHz)
        # ~3.4us of sustained activity early, so the real matmuls start warm.
        warm_w = sb.tile([64, 128], BF16)
        warm_x = sb.tile([64, 512], BF16)
        nc.gpsimd.memset(warm_w[:], 1.0)
        nc.gpsimd.memset(warm_x[:], 1.0)
        warm_ps = pk_pool.tile([128, 512], F32, tag="pk")
        for _ in range(6):
            nc.tensor.matmul(warm_ps[:], warm_w[0:64, :], warm_x[0:64, :],
                             start=True, stop=True)

        with nc.named_scope("peaky"):
            b3 = pk[0:64, 0:64]
            b33 = pk[0:64, 64:128]

            # sali = B3 @ X @ B3 (B3 symmetric banded 1/3)
            p1 = pk_pool.tile([128, 64], F32, tag="pk")
            nc.tensor.matmul(p1[:], attn[0:64, 0:128], b3, start=True, stop=True)
            s1 = sb.tile([64, 128], F32, tag="s1")
            nc.vector.memset(s1[:, 64:128], 0.0)
            nc.scalar.copy(s1[0:64, 0:64], p1[0:64, :])
            p2 = pk_pool.tile([128, 64], F32, tag="pk")
            nc.tensor.matmul(p2[:], s1[0:64, 0:128], b3, start=True, stop=True)
            sali = sb.tile([64, 128], F32, tag="sali")
            nc.vector.memset(sali[:, 64:128], 0.0)
            nc.scalar.copy(sali[0:64, 0:64], p2[0:64, :])

            # avg33(sali) = B33 @ sali @ B33
            a1p = pk_pool.tile([128, 64], F32, tag="pk")
            nc.tensor.matmul(a1p[:], sali[0:64, 0:128], b33, start=True,
                             stop=True)
            a1 = sb.tile([64, 128], F32, tag="a1")
            nc.vector.memset(a1[:, 64:128], 0.0)
            nc.scalar.copy(a1[0:64, 0:64], a1p[0:64, :])
            a2p = pk_pool.tile([128, 64], F32, tag="pk")
            nc.tensor.matmul(a2p[:], a1[0:64, 0:128], b33, start=True, stop=True)

            # max33(sali), separable via log-step chains on the otherwise-idle
            # GpSimd engine; only never-written pad columns get -inf memsets.
            def max_chain(src_ap, tag, src_psum=False):
                pb = sb.tile([64, 128], F32, tag=tag + "pb")
                nc.gpsimd.memset(pb[:, 0:16], NEG_INF)
                nc.gpsimd.memset(pb[:, 80:112], NEG_INF)
                nc.scalar.copy(pb[:, 16:80], src_ap)
                cur = pb
                for k in (1, 2, 4, 8, 16):
                    nxt = sb.tile([64, 128], F32, tag=tag + "s%d" % k)
                    nc.gpsimd.memset(nxt[:, 96:112], NEG_INF)
                    nc.vector.tensor_max(nxt[:, 0:96], cur[:, 0:96],
                                         cur[:, k:96 + k])
                    cur = nxt
                out = sb.tile([64, 64], F32, tag=tag + "o")
                nc.vector.tensor_max(out[:], cur[:, 0:64], pb[:, 32:96])
                return out

            m1 = max_chain(sali[0:64, 0:64], "mA")         # pooled along w
            m1t = sb.tile([64, 64], F32, tag="m1t")
            for bi in range(2):
                for bj in range(2):
                    nc.vector.transpose(m1t[ds(32 * bi, 32), ds(32 * bj, 32)],
                                        m1[ds(32 * bj, 32), ds(32 * bi, 32)])
            m2 = max_chain(m1t[:], "mB")                    # pooled along h too

            gapm = sb.tile([64, 2], F32)
            nc.vector.reduce_sum(gapm[:, 0:1], m2[:], axis=mybir.AxisListType.X)
            nc.vector.reduce_sum(gapm[:, 1:2], a2p[0:64, :],
                                 axis=mybir.AxisListType.X)
            nc.sync.dma_start(pkout_d, gapm[:])
        pk_scope.close()

        gram_pool = ctx.enter_context(tc.tile_pool(name="gram", bufs=1,
                                                   space="PSUM"))
        cs_pool = ctx.enter_context(tc.tile_pool(name="cs", bufs=1, space="PSUM"))
        colaccA = cs_pool.tile([128, 512], F32, tag="ca")
        colaccB = cs_pool.tile([128, 512], F32, tag="cb")

        # ---- main loop: gram + threshold + colsum, software-pipelined so the
        # PE always has the PREVIOUS pair's colsum matmuls to chew on while
        # ACT/DVE consume the current pair's gram banks ----
        ncs = [0, 0]
        n_cs = NCH * NT

        def emit_colsum(sc, p, sgn):
            for t in (2 * p, 2 * p + 1):
                for cc in range(2):
                    c = 2 * sc + cc
                    v = c * 2 + int(ASSIGN[t, sc])
                    st = sgn[:, ds(t * 1024 + cc * 512, 512)]
                    for h, acc in ((0, colaccA), (1, colaccB)):
                        ncs[h] += 1
                        nc.tensor.matmul(
                            acc[:], wsel[ds(64 * h, 64), ts(v, 128)],
                            st[ds(64 * h, 64), :],
                            start=(ncs[h] == 1), stop=(ncs[h] == n_cs),
                            skip_group_check=True)

        with nc.named_scope("main"):
            pending = None
            for sc in range(NSC):
                c0, c1 = 2 * sc, 2 * sc + 1
                sgn = sgn_pool.tile([128, NT * 1024], BF16, tag="sgn")
                for p in range(NT // 2):
                    g0 = gram_pool.tile([128, 1024], F32,
                                        tag="g%d" % ((2 * p) % 3))
                    g1 = gram_pool.tile([128, 1024], F32,
                                        tag="g%d" % ((2 * p + 1) % 3))
                    wl = d1w[0:C, ts(p, 128)]
                    wh = d1w[64:64 + C, ts(p, 128)]
                    nc.tensor.matmul(g0[:, 0:512], wl, d2d[0:C, ts(c0, 512)],
                                     start=True, stop=True)
                    nc.tensor.matmul(g0[:, 512:1024], wl, d2d[0:C, ts(c1, 512)],
                                     start=True, stop=True)
                    nc.tensor.matmul(g1[:, 0:512], wh,
                                     d2d[64:64 + C, ts(c0, 512)],
                                     start=True, stop=True)
                    nc.tensor.matmul(g1[:, 512:1024], wh,
                                     d2d[64:64 + C, ts(c1, 512)],
                                     start=True, stop=True)
                    for t, g in ((2 * p, g0), (2 * p + 1, g1)):
                        s = sgn[:, ts(t, 1024)]
                        if ASSIGN[t, sc] == 0:
                            nc.scalar.activation(
                                s, g[:], Sign, bias=nthr[:], scale=1.0,
                                accum_out=rowaccA[:, ds(t * NSC + sc, 1)])
                        else:
                            nc.vector.tensor_scalar(
                                s, g[:], THR, None,
                                mybir.AluOpType.is_gt, mybir.AluOpType.add,
                                accum_out=rowaccD[:, ds(t * NSC + sc, 1)])
                    if pending is not None:
                        emit_colsum(*pending)
                    pending = (sc, p, sgn)
            emit_colsum(*pending)

        # ---- row-count reduction + export ----
        redA = sb.tile([128, NT], F32)
        redD = sb.tile([128, NT], F32)
        nc.vector.reduce_sum(
            redA[:], rowaccA[:].rearrange("p (t c) -> p t c", t=NT, c=NSC),
            axis=mybir.AxisListType.X)
        nc.vector.reduce_sum(
            redD[:], rowaccD[:].rearrange("p (t c) -> p t c", t=NT, c=NSC),
            axis=mybir.AxisListType.X)
        nc.gpsimd.dma_start(rowA_d, redA[:])
        nc.gpsimd.dma_start(rowD_d, redD[:])

        # ---- column-count export ----
        colA_sb = sb.tile([NCH, 512], F32)
        nc.scalar.copy(colA_sb[:], colaccA[0:NCH, :])
        col_sb = sb.tile([NCH, 512], F32)
        nc.vector.tensor_add(col_sb[:], colA_sb[:], colaccB[0:NCH, :])
        nc.sync.dma_start(cols_d, col_sb[:])

    nc.compile()
    return nc


def _get_program():
    global _PROGRAM
    if _PROGRAM is None:
        _PROGRAM = _build_program()
    return _PROGRAM


def _normalize(x):
    n = np.sqrt((x * x).sum(axis=0, keepdims=True, dtype=np.float32))
    return (x / np.maximum(n, np.float32(1e-12))).astype(np.float32)


def _make_consts():
    idx = np.arange(64)
    b3 = (np.abs(idx[:, None] - idx[None, :]) <= 1).astype(np.float32) / \
        np.float32(3.0)
    b33 = (np.abs(idx[:, None] - idx[None, :]) <= 16).astype(np.float32) / \
        np.float32(33.0)
    pk = np.concatenate([b3, b33], axis=1).astype(np.float32)  # [64, 128]

    wsel = np.zeros((128, NCH * 2 * 128), np.float32)
    for c in range(NCH):
        for k in range(2):
            v = c * 2 + k
            wsel[:, v * 128 + c] = 0.5 if k == 0 else 1.0
    return pk, wsel.astype(ml_dtypes.bfloat16)


def _prepare_in_maps(x1, x2, a1, a2):
    pk, wsel = _make_consts()
    in_maps = []
    for b in range(B):
        d1 = _normalize(x1[b].reshape(C, N).astype(np.float32)) \
            .astype(ml_dtypes.bfloat16)
        d2 = _normalize(x2[b].reshape(C, N).astype(np.float32)) \
            .astype(ml_dtypes.bfloat16)
        d2d = np.zeros((128, N), ml_dtypes.bfloat16)
        d2d[0:C] = d2
        d2d[64:64 + C] = d2
        for half in range(2):
            base = half * ROWS
            d1w = np.zeros((128, NT * 64), ml_dtypes.bfloat16)
            for p in range(NT // 2):
                d1w[0:C, p * 128:(p + 1) * 128] = \
                    d1[:, base + (2 * p) * 128: base + (2 * p + 1) * 128]
                d1w[64:64 + C, p * 128:(p + 1) * 128] = \
                    d1[:, base + (2 * p + 1) * 128: base + (2 * p + 2) * 128]
            attn = (a1 if half == 0 else a2)[b, 0].astype(np.float32)
            in_maps.append({
                "d1w": d1w, "d2d": d2d, "wsel": np.asarray(wsel),
                "attn": np.ascontiguousarray(attn), "pk": pk,
            })
    return in_maps


def _postprocess(results, x1, x2, a1, a2, pos2):
    f32 = np.float32
    mx1_halves = []
    colparts = []
    gap_means = np.zeros((NCORES,), np.float32)
    for core, r in enumerate(results):
        rowA = r["rowA"].astype(np.float32)   # [128, NT]
        rowD = r["rowD"].astype(np.float32)
        cntA = (rowA + 1024.0 * N_ACT_PER_TILE[None, :]) * 0.5
        cnt = cntA + rowD                      # [128, NT] rows of this core
        mx1_halves.append(cnt.T.reshape(ROWS))  # row t*128+p
        cols = r["cols"].astype(np.float32)    # [NCH, 512]
        colpart = cols + 64.0 * N_ACT_PER_CHUNK[:, None]
        colparts.append(colpart.reshape(N))
        pkk = r["pkout"].astype(np.float32)    # [64, 2]
        gap_means[core] = (pkk[:, 0].sum() - pkk[:, 1].sum()) / f32(N)

    loss_imgs = np.zeros((B,), np.float32)
    for b in range(B):
        mx1 = np.concatenate([mx1_halves[2 * b], mx1_halves[2 * b + 1]])
        colcnt = colparts[2 * b] + colparts[2 * b + 1]
        flat2 = (pos2[b, 0].astype(np.int64) * W +
                 pos2[b, 1].astype(np.int64))
        mx2 = colcnt[flat2]
        scores1 = a1[b].reshape(N).astype(np.float32)
        scores2 = a2[b].reshape(N).astype(np.float32)[flat2]
        t1 = (f32(1.0) / (f32(1.0) + mx1.astype(np.float32)) **
              f32(TAU)).astype(np.float32)
        t2 = (f32(1.0) / (f32(1.0) + mx2.astype(np.float32)) **
              f32(TAU)).astype(np.float32)
        loss_imgs[b] = (np.abs(scores1 - t1).mean(dtype=np.float32) +
                        np.abs(scores2 - t2).mean(dtype=np.float32))

    loss = loss_imgs.mean(dtype=np.float32)
    pk1 = max(f32(0.0), f32(1.0) - gap_means[0::2].mean(dtype=np.float32))
    pk2 = max(f32(0.0), f32(1.0) - gap_means[1::2].mean(dtype=np.float32))
    loss = loss + f32(LAMBDA_PEAKY) * (pk1 + pk2) / f32(2.0)
    return np.asarray(loss, dtype=np.float32)


def _run(x1_encoded, x2_encoded, attentions1, attentions2, fmap_pos2,
         trace=False, trace_cores=None):
    from concourse import bass_utils

    nc = _get_program()
    in_maps = _prepare_in_maps(np.asarray(x1_encoded), np.asarray(x2_encoded),
                               np.asarray(attentions1),
                               np.asarray(attentions2))
    res = bass_utils.run_bass_kernel_spmd(
        nc, in_maps, core_ids=list(range(NCORES)), trace=trace,
        trace_cores=trace_cores)
    loss = _postprocess(res.results, np.asarray(x1_encoded),
                        np.asarray(x2_encoded), np.asarray(attentions1),
                        np.asarray(attentions2), np.asarray(fmap_pos2))
    return loss, res


def kernel(x1_encoded, x2_encoded, attentions1, attentions2, fmap_pos2):
    loss, _ = _run(x1_encoded, x2_encoded, attentions1, attentions2,
                   fmap_pos2)
    return loss
